# revision 11
# baseline (speedup 1.0000x reference)
"""Trainium2 Bass kernel for an AQT quantized Dense layer.

Host pre-quantizes x to integer values (exact AQT rounding) and ships them
as bf16 (integers <= 127 are exact in bf16), halving the x DMA traffic and
removing all device-side quantize ops. The dequantized weight matrix
(0.5MB, bf16) is also computed on host and shipped in the matmul-ready
layout. Output is written as bf16. The device is then a pure bf16-matmul
pipeline bound by the TensorEngine: DMA in (2 queues) -> PE matmul ->
PSUM->SBUF bf16 copy (alternating ACT/DVE) -> DMA out (2 queues).
"""

import numpy as np

B, D, F = 131072, 512, 512
NCORES = 8
BS = B // NCORES           # rows per core
P = 128                    # partitions
KC = D // P                # contraction chunks
LB = 512                   # load block: b-rows per x DMA tile (4KB runs)
NLB = BS // LB             # load blocks per core
JT = LB // P               # b-chunks of 128 rows per load block
SB = 1024                  # store superblock: b-rows per y DMA (8KB runs)
NSB = BS // SB
JC = SB // P

A_SCALE = float(np.float32(127.0 / 6.0))
EPS = 1e-6

_NC_CACHE = {}


def _build_nc():
    import concourse.bacc as bacc
    import concourse.mybir as mybir
    import concourse.tile as tile

    f32 = mybir.dt.float32
    bf16 = mybir.dt.bfloat16

    nc = bacc.Bacc("TRN2", target_bir_lowering=False, debug=False,
                   enable_asserts=False)
    x_t = nc.dram_tensor("xt", [NLB, P, KC, LB], bf16, kind="ExternalInput")
    w_t = nc.dram_tensor("wd", [P, KC, F], bf16, kind="ExternalInput")
    y_t = nc.dram_tensor("out", [NSB, P, JC, F], bf16, kind="ExternalOutput")
    x_ap, w_ap, y_ap = x_t.ap(), w_t.ap(), y_t.ap()

    with tile.TileContext(nc) as tc:
        from contextlib import ExitStack
        with ExitStack() as ctx:
            wpool = ctx.enter_context(tc.tile_pool(name="wdeq", bufs=1))
            xin = ctx.enter_context(tc.tile_pool(name="xin", bufs=5))
            yout = ctx.enter_context(tc.tile_pool(name="yout", bufs=3))
            mmps = ctx.enter_context(tc.tile_pool(name="mmps", bufs=8,
                                                  space="PSUM"))

            # dequantized weights arrive ready-to-use; single DMA, 4KB runs
            # on a hardware DGE queue, issued before any x-load descriptor
            wdt = wpool.tile([P, KC, F], bf16, tag="wdeq")
            nc.sync.dma_start(out=wdt, in_=w_ap)

            # warm the PE p-state during the initial DMA latency: the clock
            # ramps 1.2->2.4GHz after ~3us of sustained matmul activity, so
            # burn that ramp on throwaway matmuls instead of the real stream
            warm = wpool.tile([P, F], bf16, tag="warm")
            nc.gpsimd.memset(warm, 0.0)
            wps = mmps.tile([P, F], f32, tag="yp")
            for _ in range(20):
                nc.tensor.matmul(wps, warm[:, 0:P], warm,
                                 start=True, stop=True)

            yf = None
            for s in range(NLB):
                # 512KB load, 4KB per-partition runs, alternating queues
                xf = xin.tile([P, KC, LB], bf16, tag="xf")
                if s % 2 == 0:
                    nc.sync.dma_start(out=xf, in_=x_ap[s])
                else:
                    nc.scalar.dma_start(out=xf, in_=x_ap[s])
                if s % 2 == 0:
                    yf = yout.tile([P, JC, F], bf16, tag="yf")
                for jj in range(JT):
                    j = (s % 2) * JT + jj
                    yp = mmps.tile([P, F], f32, tag="yp")
                    for k in range(KC):
                        nc.tensor.matmul(yp,
                                         xf[:, k, jj * P:(jj + 1) * P],
                                         wdt[:, k, :],
                                         start=(k == 0), stop=(k == KC - 1))
                    # PSUM -> SBUF bf16, alternating engines to stay off
                    # the PE critical path
                    if j % 2 == 0:
                        nc.scalar.copy(yf[:, j, :], yp)
                    else:
                        nc.vector.tensor_copy(yf[:, j, :], yp)
                        if s >= NLB - 2:
                            # final superblock: stream per-j-pair stores so
                            # the last transfer after the last copy is small
                            with tc.high_priority():
                                eng = nc.sync if (j // 2) % 2 == 0 \
                                    else nc.scalar
                                eng.dma_start(
                                    out=y_ap[s // 2, :, j - 1:j + 1, :],
                                    in_=yf[:, j - 1:j + 1, :])
                if s % 2 == 1 and s != NLB - 1:
                    # 1MB store, 8KB per-partition runs, alternating HW queues
                    with tc.high_priority():
                        eng = nc.sync if (s // 2) % 2 == 0 else nc.scalar
                        eng.dma_start(out=y_ap[s // 2], in_=yf)

    nc.compile()
    return nc


def _get_nc():
    if "nc" not in _NC_CACHE:
        _NC_CACHE["nc"] = _build_nc()
    return _NC_CACHE["nc"]


def kernel(**inputs):
    import ml_dtypes
    from concourse.bass_utils import run_bass_kernel_spmd

    x = np.asarray(inputs["x"], dtype=np.float32)
    kern = np.asarray(inputs["kernel"], dtype=np.float32)

    # AQT weight quantization + dequantization on host (0.5MB shipped)
    w_bound = np.maximum(np.abs(kern).max(axis=0, keepdims=True),
                         np.float32(EPS))
    w_scale = np.float32(127.0) / w_bound
    w_q = np.clip(np.rint(kern * w_scale), -127.0, 127.0)
    w_deq = (w_q * (w_bound / np.float32(127.0) / np.float32(A_SCALE)))
    # layout [P, KC, F]: wd[p, k, f] = w_deq[k*128 + p, f]
    wd = np.ascontiguousarray(
        w_deq.astype(ml_dtypes.bfloat16).reshape(KC, P, F).transpose(1, 0, 2))

    # exact AQT activation quantization on host; integer values <= 127 are
    # exactly representable in bf16
    xq = np.clip(np.rint(x * np.float32(A_SCALE)), -127.0, 127.0)
    xb = xq.astype(ml_dtypes.bfloat16)
    # packed layout: [NLB, P, KC, LB]; xtile[s, p, c, b] = x[s*LB+b, c*P+p]
    shards = [np.ascontiguousarray(
                  xb[i * BS:(i + 1) * BS].reshape(NLB, LB, KC, P)
                  .transpose(0, 3, 2, 1))
              for i in range(NCORES)]

    nc = _get_nc()
    in_maps = [{"xt": s, "wd": wd} for s in shards]
    res = run_bass_kernel_spmd(nc, in_maps, core_ids=list(range(NCORES)))
    # un-tile: y[b0+128j+p, f] = y_tiled[s, p, j, f]
    out = np.concatenate(
        [r["out"].astype(np.float32).transpose(0, 2, 1, 3).reshape(BS, F)
         for r in res.results],
        axis=0)
    out = np.ascontiguousarray(out)

    bias = inputs.get("bias")
    if bias is not None and np.any(np.asarray(bias)):
        out = out + np.asarray(bias, dtype=np.float32)[None, :]
    return out


# revision 12
# speedup vs baseline: 1.0164x; 1.0164x over previous
"""Trainium2 Bass kernel for an AQT quantized Dense layer.

Host pre-quantizes x to integer values (exact AQT rounding) and ships them
as bf16 (integers <= 127 are exact in bf16), halving the x DMA traffic and
removing all device-side quantize ops. The dequantized weight matrix
(0.5MB, bf16) is also computed on host and shipped in the matmul-ready
layout. Output is written as bf16. The device is then a pure bf16-matmul
pipeline bound by the TensorEngine: DMA in (2 queues) -> PE matmul ->
PSUM->SBUF bf16 copy (alternating ACT/DVE) -> DMA out (2 queues).
"""

import numpy as np

B, D, F = 131072, 512, 512
NCORES = 8
BS = B // NCORES           # rows per core
P = 128                    # partitions
KC = D // P                # contraction chunks
LB = 512                   # load block: b-rows per x DMA tile (4KB runs)
NLB = BS // LB             # load blocks per core
JT = LB // P               # b-chunks of 128 rows per load block
SB = 1024                  # store superblock: b-rows per y DMA (8KB runs)
NSB = BS // SB
JC = SB // P

A_SCALE = float(np.float32(127.0 / 6.0))
EPS = 1e-6

_NC_CACHE = {}


def _build_nc():
    import concourse.bacc as bacc
    import concourse.mybir as mybir
    import concourse.tile as tile

    f32 = mybir.dt.float32
    bf16 = mybir.dt.bfloat16

    nc = bacc.Bacc("TRN2", target_bir_lowering=False, debug=False,
                   enable_asserts=False)
    x_t = nc.dram_tensor("xt", [NLB, P, KC, LB], bf16, kind="ExternalInput")
    w_t = nc.dram_tensor("wd", [P, KC, F], bf16, kind="ExternalInput")
    y_t = nc.dram_tensor("out", [NSB, P, JC, F], bf16, kind="ExternalOutput")
    x_ap, w_ap, y_ap = x_t.ap(), w_t.ap(), y_t.ap()

    with tile.TileContext(nc) as tc:
        from contextlib import ExitStack
        with ExitStack() as ctx:
            wpool = ctx.enter_context(tc.tile_pool(name="wdeq", bufs=1))
            xin = ctx.enter_context(tc.tile_pool(name="xin", bufs=5))
            yout = ctx.enter_context(tc.tile_pool(name="yout", bufs=3))
            mmps = ctx.enter_context(tc.tile_pool(name="mmps", bufs=8,
                                                  space="PSUM"))

            # dequantized weights arrive ready-to-use; single DMA, 4KB runs
            # on a hardware DGE queue, issued before any x-load descriptor
            wdt = wpool.tile([P, KC, F], bf16, tag="wdeq")
            nc.scalar.dma_start(out=wdt, in_=w_ap)

            # warm the PE p-state during the initial DMA latency: the clock
            # ramps 1.2->2.4GHz after ~3us of sustained matmul activity, so
            # burn that ramp on throwaway matmuls instead of the real stream
            warm = wpool.tile([P, F], bf16, tag="warm")
            nc.gpsimd.memset(warm, 0.0)
            wps = mmps.tile([P, F], f32, tag="yp")
            for _ in range(12):
                nc.tensor.matmul(wps, warm[:, 0:P], warm,
                                 start=True, stop=True)

            yf = None
            for s in range(NLB):
                # 512KB load, 4KB per-partition runs, alternating queues
                xf = xin.tile([P, KC, LB], bf16, tag="xf")
                if s % 2 == 0:
                    nc.sync.dma_start(out=xf, in_=x_ap[s])
                else:
                    nc.scalar.dma_start(out=xf, in_=x_ap[s])
                if s % 2 == 0:
                    yf = yout.tile([P, JC, F], bf16, tag="yf")
                for jj in range(JT):
                    j = (s % 2) * JT + jj
                    yp = mmps.tile([P, F], f32, tag="yp")
                    for k in range(KC):
                        nc.tensor.matmul(yp,
                                         xf[:, k, jj * P:(jj + 1) * P],
                                         wdt[:, k, :],
                                         start=(k == 0), stop=(k == KC - 1))
                    # PSUM -> SBUF bf16, alternating engines to stay off
                    # the PE critical path
                    if j % 2 == 0:
                        nc.scalar.copy(yf[:, j, :], yp)
                    else:
                        nc.vector.tensor_copy(yf[:, j, :], yp)
                        if s >= NLB - 2:
                            # final superblock: stream per-j-pair stores so
                            # the last transfer after the last copy is small
                            with tc.high_priority():
                                eng = nc.sync if (j // 2) % 2 == 0 \
                                    else nc.scalar
                                eng.dma_start(
                                    out=y_ap[s // 2, :, j - 1:j + 1, :],
                                    in_=yf[:, j - 1:j + 1, :])
                if s % 2 == 1 and s != NLB - 1:
                    # 1MB store, 8KB per-partition runs, alternating HW queues
                    with tc.high_priority():
                        eng = nc.sync if (s // 2) % 2 == 0 else nc.scalar
                        eng.dma_start(out=y_ap[s // 2], in_=yf)

    nc.compile()
    return nc


def _get_nc():
    if "nc" not in _NC_CACHE:
        _NC_CACHE["nc"] = _build_nc()
    return _NC_CACHE["nc"]


def kernel(**inputs):
    import ml_dtypes
    from concourse.bass_utils import run_bass_kernel_spmd

    x = np.asarray(inputs["x"], dtype=np.float32)
    kern = np.asarray(inputs["kernel"], dtype=np.float32)

    # AQT weight quantization + dequantization on host (0.5MB shipped)
    w_bound = np.maximum(np.abs(kern).max(axis=0, keepdims=True),
                         np.float32(EPS))
    w_scale = np.float32(127.0) / w_bound
    w_q = np.clip(np.rint(kern * w_scale), -127.0, 127.0)
    w_deq = (w_q * (w_bound / np.float32(127.0) / np.float32(A_SCALE)))
    # layout [P, KC, F]: wd[p, k, f] = w_deq[k*128 + p, f]
    wd = np.ascontiguousarray(
        w_deq.astype(ml_dtypes.bfloat16).reshape(KC, P, F).transpose(1, 0, 2))

    # exact AQT activation quantization on host; integer values <= 127 are
    # exactly representable in bf16
    xq = np.clip(np.rint(x * np.float32(A_SCALE)), -127.0, 127.0)
    xb = xq.astype(ml_dtypes.bfloat16)
    # packed layout: [NLB, P, KC, LB]; xtile[s, p, c, b] = x[s*LB+b, c*P+p]
    shards = [np.ascontiguousarray(
                  xb[i * BS:(i + 1) * BS].reshape(NLB, LB, KC, P)
                  .transpose(0, 3, 2, 1))
              for i in range(NCORES)]

    nc = _get_nc()
    in_maps = [{"xt": s, "wd": wd} for s in shards]
    res = run_bass_kernel_spmd(nc, in_maps, core_ids=list(range(NCORES)))
    # un-tile: y[b0+128j+p, f] = y_tiled[s, p, j, f]
    out = np.concatenate(
        [r["out"].astype(np.float32).transpose(0, 2, 1, 3).reshape(BS, F)
         for r in res.results],
        axis=0)
    out = np.ascontiguousarray(out)

    bias = inputs.get("bias")
    if bias is not None and np.any(np.asarray(bias)):
        out = out + np.asarray(bias, dtype=np.float32)[None, :]
    return out
